# revision 8
# baseline (speedup 1.0000x reference)
"""Trainium2 Bass kernel for nn_Attention_927712936452.

Two-branch attention (self branch over x, cross branch of y-queries over
concat(x,y) keys/values), QKV + output projection, H=12 heads of 64.

Distribution: pure data-parallel over batch B=8 across the 8 NeuronCores
(one batch element per core, weights replicated). No collectives.

Per-core layout strategy (all matmul operands bf16, fp32 PSUM accumulate):
  - host supplies z^T [C, S] (c on partitions) so QKV needs no device transpose
  - stage 1 produces q^T/k^T [C, S] (head-pair per 128-row tile) and
    v natural [S, C]
  - scores are computed directly in [k, q] layout: S_T = (k^T)^T-style
    matmul with K=dh=64 contraction; head pair packed into PE row halves
  - softmax skips max-subtraction (scores ~ N(0,1) after 1/8 scale;
    exp is safe in fp32) ; exp on ScalarE with fused scale reads the
    whole [128, 1024] PSUM pair-tile in one instruction
  - row-sums come from M=1 ones-matmuls placed on otherwise-idle PE
    column strips; reciprocal on VectorE stays lane-aligned; partition
    broadcast of the reciprocal row is done by SBUF->SBUF DMA
  - normalized att^T tiles feed the output projection directly as lhsT
"""

import numpy as np

try:
    import concourse.bass as bass  # noqa: F401
except ImportError:
    import sys

    sys.path.insert(0, "/opt/trn_rl_repo")

import ml_dtypes
from contextlib import ExitStack

import concourse.bass as bass
import concourse.tile as tile
from concourse import bacc, bass_utils, mybir

BF = mybir.dt.bfloat16
F32 = mybir.dt.float32
EXP = mybir.ActivationFunctionType.Exp

# Full-size problem constants
B = 8
N_FULL = 1024  # x sequence length (self branch queries/keys)
L_FULL = 1024  # y sequence length (cross branch queries)
C_FULL = 768
H_FULL = 12
DH = 64


def build_nc(C=C_FULL, N=N_FULL, L=L_FULL, qw=512):
    """Build the per-core Bass graph.

    C: model dim (multiple of 128, heads = C//64, head pairs = C//128)
    N: x length, L: y length (each a multiple of qw; qw multiple of 128)
    """
    S = N + L
    CT = C // 128  # head-pair tiles / c-tiles
    NKT = S // 128  # k tiles over full sequence
    NKT_SELF = N // 128  # k tiles for self branch
    CH = C // 2  # proj N-split (PSUM bank limit: <=512 fp32)
    assert CH <= 512 and qw % 128 == 0 and N % qw == 0 and L % qw == 0
    scale = DH ** -0.5

    nc = bacc.Bacc("TRN2", target_bir_lowering=False, debug=False)
    zt_d = nc.dram_tensor("z_t", [C, S], BF, kind="ExternalInput")
    wq_d = nc.dram_tensor("qkv_wt", [C, 3 * C], BF, kind="ExternalInput")
    pw_d = nc.dram_tensor("proj_wt", [C, C], BF, kind="ExternalInput")
    pb_d = nc.dram_tensor("proj_b", [1, C], F32, kind="ExternalInput")
    xo_d = nc.dram_tensor("x_out", [N, C], F32, kind="ExternalOutput")
    yo_d = nc.dram_tensor("y_out", [L, C], F32, kind="ExternalOutput")

    with tile.TileContext(nc) as tc, ExitStack() as ctx:
        zt_p = ctx.enter_context(tc.tile_pool(name="zt", bufs=CT))
        wq_p = ctx.enter_context(tc.tile_pool(name="wq", bufs=CT))
        qt_p = ctx.enter_context(tc.tile_pool(name="qt", bufs=CT))
        kt_p = ctx.enter_context(tc.tile_pool(name="kt", bufs=CT))
        v_p = ctx.enter_context(tc.tile_pool(name="v", bufs=NKT))
        pw_p = ctx.enter_context(tc.tile_pool(name="pw", bufs=CT))
        misc_p = ctx.enter_context(tc.tile_pool(name="misc", bufs=1))
        p2_p = ctx.enter_context(tc.tile_pool(name="p2", bufs=3))
        att_p = ctx.enter_context(tc.tile_pool(name="attq", bufs=2 * CT))
        rr_p = ctx.enter_context(tc.tile_pool(name="rr", bufs=2))
        rb_p = ctx.enter_context(tc.tile_pool(name="rb", bufs=2))
        out_p = ctx.enter_context(tc.tile_pool(name="osb", bufs=2))
        spsum = ctx.enter_context(tc.tile_pool(name="spsum", bufs=2, space="PSUM"))
        apsum = ctx.enter_context(tc.tile_pool(name="apsum", bufs=4, space="PSUM"))
        dram_p = ctx.enter_context(tc.tile_pool(name="dstage", bufs=2, space="DRAM"))

        # ---- input loads ----
        zt = []
        wq = []
        for c in range(CT):
            z1 = zt_p.tile([128, S], BF, tag="zt")
            nc.sync.dma_start(z1[:], zt_d.ap()[c * 128 : (c + 1) * 128, :])
            zt.append(z1)
            w1 = wq_p.tile([128, 3 * C], BF, tag="wq")
            nc.sync.dma_start(w1[:], wq_d.ap()[c * 128 : (c + 1) * 128, :])
            wq.append(w1)
        pw = []
        for c in range(CT):
            p1 = pw_p.tile([128, C], BF, tag="pw")
            nc.sync.dma_start(p1[:], pw_d.ap()[c * 128 : (c + 1) * 128, :])
            pw.append(p1)
        bias = misc_p.tile([128, C], F32, tag="bias")
        nc.sync.dma_start(bias[:], pb_d.ap().to_broadcast((128, C)))
        ones_col = misc_p.tile([128, 1], BF, tag="ones")
        nc.vector.memset(ones_col[:], 1.0)

        # ---- stage 1: V = z @ Wv   (natural [s, c] layout) ----
        v_sb = [v_p.tile([128, C], BF, tag="v", name=f"v{i}") for i in range(NKT)]
        for st in range(NKT):
            for vn in range(2):
                ps = apsum.tile([128, CH], F32, tag="acc")
                for c in range(CT):
                    nc.tensor.matmul(
                        ps[:],
                        zt[c][:, st * 128 : (st + 1) * 128],
                        wq[c][:, 2 * C + vn * CH : 2 * C + (vn + 1) * CH],
                        start=(c == 0),
                        stop=(c == CT - 1),
                    )
                nc.vector.tensor_copy(v_sb[st][:, vn * CH : (vn + 1) * CH], ps[:])

        # ---- stage 1: K^T, Q^T  ([d, s] layout, head pair per tile) ----
        qtt = [qt_p.tile([128, S], BF, tag="qt", name=f"qtt{i}") for i in range(CT)]
        ktt = [kt_p.tile([128, S], BF, tag="kt", name=f"ktt{i}") for i in range(CT)]
        for t in range(CT):
            for n in range(S // 512):
                for dst, dbase in ((ktt, C), (qtt, 0)):
                    ps = apsum.tile([128, 512], F32, tag="acc")
                    for c in range(CT):
                        nc.tensor.matmul(
                            ps[:],
                            wq[c][:, dbase + t * 128 : dbase + (t + 1) * 128],
                            zt[c][:, n * 512 : (n + 1) * 512],
                            start=(c == 0),
                            stop=(c == CT - 1),
                        )
                    nc.vector.tensor_copy(dst[t][:, n * 512 : (n + 1) * 512], ps[:])

        # ---- attention + projection ----
        for branch in (0, 1):
            nkt = NKT_SELF if branch == 0 else NKT
            qbase = 0 if branch == 0 else N
            out_d = xo_d if branch == 0 else yo_d
            nq = (N if branch == 0 else L) // qw
            for qt2 in range(nq):
                qoff = qbase + qt2 * qw
                attT = [att_p.tile([128, qw], BF, tag="attT", name=f"attT{i}") for i in range(CT)]
                for hp in range(CT):
                    accA = apsum.tile([128, qw], F32, tag="acc")
                    accB = apsum.tile([128, qw], F32, tag="acc")
                    for kt in range(nkt):
                        s2 = spsum.tile([128, 2 * qw], F32, tag="s2")
                        nc.tensor.matmul(
                            s2[:, 0:qw],
                            ktt[hp][0:64, kt * 128 : (kt + 1) * 128],
                            qtt[hp][0:64, qoff : qoff + qw],
                            start=True,
                            stop=True,
                        )
                        nc.tensor.matmul(
                            s2[:, qw : 2 * qw],
                            ktt[hp][64:128, kt * 128 : (kt + 1) * 128],
                            qtt[hp][64:128, qoff : qoff + qw],
                            start=True,
                            stop=True,
                            tile_position=(64, 0),
                        )
                        p2 = p2_p.tile([128, 2 * qw], BF, tag="p2")
                        nc.scalar.activation(p2[:], s2[:], EXP, scale=scale)
                        first = kt == 0
                        last = kt == nkt - 1
                        nc.tensor.matmul(
                            accA[0:64, :],
                            v_sb[kt][:, (2 * hp) * 64 : (2 * hp + 1) * 64],
                            p2[:, 0:qw],
                            start=first,
                            stop=last,
                            skip_group_check=True,
                        )
                        nc.tensor.matmul(
                            accB[64:128, :],
                            v_sb[kt][:, (2 * hp + 1) * 64 : (2 * hp + 2) * 64],
                            p2[:, qw : 2 * qw],
                            start=first,
                            stop=last,
                            tile_position=(0, 64),
                            skip_group_check=True,
                        )
                        nc.tensor.matmul(
                            accA[64:65, :],
                            ones_col[:],
                            p2[:, 0:qw],
                            start=first,
                            stop=last,
                            tile_position=(0, 64),
                            skip_group_check=True,
                        )
                        nc.tensor.matmul(
                            accB[0:1, :],
                            ones_col[:],
                            p2[:, qw : 2 * qw],
                            start=first,
                            stop=last,
                            skip_group_check=True,
                        )
                    rr = rr_p.tile([128, qw], F32, tag="rr")
                    nc.vector.reciprocal(rr[64:65, :], accA[64:65, :])
                    nc.vector.reciprocal(rr[0:1, :], accB[0:1, :])
                    # partition-broadcast of the reciprocal rows: SBUF-source
                    # step-0 DMAs are rejected, so bounce via a DRAM row
                    rs_d = dram_p.tile([2, qw], F32, tag="rsd")
                    nc.sync.dma_start(rs_d[0:1, :], rr[64:65, :])
                    nc.sync.dma_start(rs_d[1:2, :], rr[0:1, :])
                    rb = rb_p.tile([128, qw], F32, tag="rb")
                    nc.sync.dma_start(rb[0:64, :], rs_d[0:1, :].to_broadcast((64, qw)))
                    nc.sync.dma_start(
                        rb[64:128, :], rs_d[1:2, :].to_broadcast((64, qw))
                    )
                    nc.vector.tensor_mul(attT[hp][0:64, :], accA[0:64, :], rb[0:64, :])
                    nc.vector.tensor_mul(
                        attT[hp][64:128, :], accB[64:128, :], rb[64:128, :]
                    )
                # projection for this q block
                for lt in range(qw // 128):
                    osb = out_p.tile([128, C], F32, tag="osb")
                    for half in range(2):
                        pp = apsum.tile([128, CH], F32, tag="acc")
                        for ct in range(CT):
                            nc.tensor.matmul(
                                pp[:],
                                attT[ct][:, lt * 128 : (lt + 1) * 128],
                                pw[ct][:, half * CH : (half + 1) * CH],
                                start=(ct == 0),
                                stop=(ct == CT - 1),
                            )
                        nc.vector.tensor_add(
                            osb[:, half * CH : (half + 1) * CH],
                            pp[:],
                            bias[:, half * CH : (half + 1) * CH],
                        )
                    row0 = qt2 * qw + lt * 128
                    nc.sync.dma_start(out_d.ap()[row0 : row0 + 128, :], osb[:])

    nc.compile()
    return nc


def _prep_core_inputs(xb, yb, qkv_wt_bf, proj_wt_bf, proj_b):
    z = np.concatenate([xb, yb], axis=0)  # [S, C]
    zt = np.ascontiguousarray(z.T).astype(ml_dtypes.bfloat16)
    return {
        "z_t": zt,
        "qkv_wt": qkv_wt_bf,
        "proj_wt": proj_wt_bf,
        "proj_b": proj_b.reshape(1, -1).astype(np.float32),
    }


def kernel(x, y, qkv_w, proj_w, proj_b):
    x = np.asarray(x, dtype=np.float32)
    y = np.asarray(y, dtype=np.float32)
    qkv_w = np.asarray(qkv_w, dtype=np.float32)
    proj_w = np.asarray(proj_w, dtype=np.float32)
    proj_b = np.asarray(proj_b, dtype=np.float32)

    qkv_wt_bf = np.ascontiguousarray(qkv_w.T).astype(ml_dtypes.bfloat16)
    proj_wt_bf = np.ascontiguousarray(proj_w.T).astype(ml_dtypes.bfloat16)

    in_maps = [
        _prep_core_inputs(x[b], y[b], qkv_wt_bf, proj_wt_bf, proj_b)
        for b in range(x.shape[0])
    ]
    nc = build_nc()
    res = bass_utils.run_bass_kernel_spmd(nc, in_maps, core_ids=list(range(len(in_maps))))
    x_out = np.stack([res.results[b]["x_out"] for b in range(len(in_maps))])
    y_out = np.stack([res.results[b]["y_out"] for b in range(len(in_maps))])
    return (x_out, y_out)


if __name__ == "__main__":
    import reference

    inputs = {k: np.asarray(v) for k, v in reference.setup_inputs().items()}
    out = kernel(**inputs)
    print("x_out", out[0].shape, "y_out", out[1].shape)


# revision 31
# speedup vs baseline: 9430.8216x; 9430.8216x over previous
"""Trainium2 Bass kernel for nn_Attention_927712936452.

Two-branch attention (self branch over x, cross branch of y-queries over
concat(x,y) keys/values), QKV + output projection, H=12 heads of 64.

Distribution: pure data-parallel over batch B=8 across the 8 NeuronCores
(one batch element per core, weights replicated). No collectives.

Per-core layout strategy (all matmul operands bf16, fp32 PSUM accumulate):
  - host supplies z^T [C, S] (c on partitions) so QKV needs no device transpose
  - stage 1 produces q^T/k^T [C, S] (head-pair per 128-row tile) and
    v natural [S, C]
  - scores are computed directly in [k, q] layout: S_T = (k^T)^T-style
    matmul with K=dh=64 contraction; head pair packed into PE row halves
  - softmax skips max-subtraction (scores ~ N(0,1) after 1/8 scale;
    exp is safe in fp32) ; exp on ScalarE with fused scale reads the
    whole [128, 1024] PSUM pair-tile in one instruction
  - softmax denominators ride along in the AV matmuls: v is stored as
    65-wide [v_h | 1] blocks so the M=65 AV matmul leaves the row-sum in
    accumulator row 64; reciprocal on VectorE stays lane-aligned and its
    partition-broadcast bounces through a DRAM row (SBUF-source step-0
    DMAs are rejected); head B's normalized rows are DMA-moved into the
    upper half of the att^T tile
  - normalized att^T tiles feed the output projection directly as lhsT
  - emission order: V first, then the two cross-branch q-blocks
    interleaved at head-pair granularity with the remaining stage-1 Q/K
    chains spread just-in-time across them, then the self branch
"""

import numpy as np

try:
    import concourse.bass as bass  # noqa: F401
except ImportError:
    import sys

    sys.path.insert(0, "/opt/trn_rl_repo")

import ml_dtypes
from contextlib import ExitStack

import concourse.bass as bass
import concourse.tile as tile
from concourse import bacc, bass_utils, mybir

BF = mybir.dt.bfloat16
F32 = mybir.dt.float32
EXP = mybir.ActivationFunctionType.Exp

# Full-size problem constants
B = 8
N_FULL = 1024  # x sequence length (self branch queries/keys)
L_FULL = 1024  # y sequence length (cross branch queries)
C_FULL = 768
H_FULL = 12
DH = 64


def build_nc(C=C_FULL, N=N_FULL, L=L_FULL, qw=512, ablate=(), small_out=False):
    """Build the per-core Bass graph.

    C: model dim (multiple of 128, heads = C//64, head pairs = C//128)
    N: x length, L: y length (each a multiple of qw; qw multiple of 128)
    """
    S = N + L
    CT = C // 128  # head-pair tiles / c-tiles
    NKT = S // 128  # k tiles over full sequence
    NKT_SELF = N // 128  # k tiles for self branch
    CH = C // 2  # proj N-split (PSUM bank limit: <=512 fp32)
    assert CH <= 512 and qw % 128 == 0 and N % qw == 0 and L % qw == 0
    scale = DH ** -0.5

    nc = bacc.Bacc("TRN2", target_bir_lowering=False, debug=False)
    zt_d = nc.dram_tensor("z_t", [C, S], BF, kind="ExternalInput")
    wq_d = nc.dram_tensor("qkv_wt", [C, 3 * C], BF, kind="ExternalInput")
    pw_d = nc.dram_tensor("proj_wt", [C, C], BF, kind="ExternalInput")
    pb_d = nc.dram_tensor("proj_b", [1, C], F32, kind="ExternalInput")
    on = 128 if small_out else N
    ol = 128 if small_out else L
    xo_d = nc.dram_tensor("x_out", [on, C], F32, kind="ExternalOutput")
    yo_d = nc.dram_tensor("y_out", [ol, C], F32, kind="ExternalOutput")

    with tile.TileContext(nc) as tc, ExitStack() as ctx:
        zt_p = ctx.enter_context(tc.tile_pool(name="zt", bufs=CT))
        wq_p = ctx.enter_context(tc.tile_pool(name="wq", bufs=CT))
        qt_p = ctx.enter_context(tc.tile_pool(name="qt", bufs=CT))
        kt_p = ctx.enter_context(tc.tile_pool(name="kt", bufs=CT))
        v_p = ctx.enter_context(tc.tile_pool(name="v", bufs=NKT))
        pw_p = ctx.enter_context(tc.tile_pool(name="pw", bufs=CT))
        misc_p = ctx.enter_context(tc.tile_pool(name="misc", bufs=1))
        p2_p = ctx.enter_context(tc.tile_pool(name="p2", bufs=6))
        att_p = ctx.enter_context(tc.tile_pool(name="attq", bufs=3 * CT))
        rr_p = ctx.enter_context(tc.tile_pool(name="rr", bufs=4))
        rb_p = ctx.enter_context(tc.tile_pool(name="rb", bufs=4))
        out_p = ctx.enter_context(tc.tile_pool(name="osb", bufs=3))
        spsum = ctx.enter_context(tc.tile_pool(name="spsum", bufs=2, space="PSUM"))
        apsum = ctx.enter_context(tc.tile_pool(name="apsum", bufs=4, space="PSUM"))
        dram_p = ctx.enter_context(tc.tile_pool(name="dstage", bufs=2, space="DRAM"))

        # ---- input loads ----
        # chunked so the first V chains / attention can start before the
        # whole input is resident
        zt = [zt_p.tile([128, S], BF, tag="zt", name=f"zt{c}") for c in range(CT)]
        wq = [wq_p.tile([128, 3 * C], BF, tag="wq", name=f"wqt{c}") for c in range(CT)]
        ZCH = S // 4
        for ch in range(4):
            for c in range(CT):
                nc.sync.dma_start(
                    zt[c][:, ch * ZCH : (ch + 1) * ZCH],
                    zt_d.ap()[c * 128 : (c + 1) * 128, ch * ZCH : (ch + 1) * ZCH],
                )
            if ch == 0:
                for c in range(CT):
                    nc.sync.dma_start(
                        wq[c][:, 2 * C : 3 * C],
                        wq_d.ap()[c * 128 : (c + 1) * 128, 2 * C : 3 * C],
                    )
        for c in range(CT):
            nc.sync.dma_start(
                wq[c][:, 0 : 2 * C], wq_d.ap()[c * 128 : (c + 1) * 128, 0 : 2 * C]
            )
        pw = []
        for c in range(CT):
            p1 = pw_p.tile([128, C], BF, tag="pw")
            nc.sync.dma_start(p1[:], pw_d.ap()[c * 128 : (c + 1) * 128, :])
            pw.append(p1)
        bias = misc_p.tile([128, C], F32, tag="bias")
        nc.sync.dma_start(bias[:], pb_d.ap().to_broadcast((128, C)))

        # ---- stage 1: V = z @ Wv ----
        # v-hat layout: every head gets a 65-wide block [v_h | 1]; the ones
        # column lets even heads' AV matmul (M=65) produce the softmax
        # denominator for free in row 64 of the accumulator.
        H = C // DH
        HH = H // 2  # heads per CH half
        v_sb = [v_p.tile([128, H * 65], BF, tag="v", name=f"v{i}") for i in range(NKT)]
        v_emitted = set()

        def emit_v(st):
            if st in v_emitted:
                return
            v_emitted.add(st)
            vh3 = v_sb[st][:].rearrange("p (h e) -> p h e", e=65)
            nc.vector.memset(vh3[:, :, 64:65], 1.0)
            for vn in range(2):
                ps = apsum.tile([128, CH], F32, tag="acc", name=f"vps{st}_{vn}")
                for c in range(CT):
                    nc.tensor.matmul(
                        ps[:],
                        zt[c][:, st * 128 : (st + 1) * 128],
                        wq[c][:, 2 * C + vn * CH : 2 * C + (vn + 1) * CH],
                        start=(c == 0),
                        stop=(c == CT - 1),
                    )
                nc.vector.tensor_copy(
                    vh3[:, vn * HH : (vn + 1) * HH, 0:64],
                    ps[:].rearrange("p (h e) -> p h e", e=64),
                )

        # ---- stage 1: K^T, Q^T  ([d, s] layout, head pair per tile) ----
        qtt = [qt_p.tile([128, S], BF, tag="qt", name=f"qtt{i}") for i in range(CT)]
        ktt = [kt_p.tile([128, S], BF, tag="kt", name=f"ktt{i}") for i in range(CT)]

        def emit_kq_chain(t, n, dst, dbase):
            ps = apsum.tile([128, 512], F32, tag="acc", name=f"kq{t}_{n}_{dbase}")
            for c in range(CT):
                nc.tensor.matmul(
                    ps[:],
                    wq[c][:, dbase + t * 128 : dbase + (t + 1) * 128],
                    zt[c][:, n * 512 : (n + 1) * 512],
                    start=(c == 0),
                    stop=(c == CT - 1),
                )
            nc.vector.tensor_copy(dst[t][:, n * 512 : (n + 1) * 512], ps[:])

        def kq_chain_thunks(t):
            return [
                (lambda t=t, n=n, dst=dst, dbase=dbase: emit_kq_chain(t, n, dst, dbase))
                for n in range(S // 512)
                for dst, dbase in ((ktt, C), (qtt, 0))
            ]

        def emit_kq(t):
            for th in kq_chain_thunks(t):
                th()

        # ---- attention + projection ----
        def emit_attn_hp(branch, qt2, hp, attT, filler=()):
            filler = list(filler)
            nkt = NKT_SELF if branch == 0 else NKT
            qbase = 0 if branch == 0 else N
            qoff = qbase + qt2 * qw
            accA = apsum.tile([128, qw], F32, tag="acc", name=f"accA{branch}{qt2}{hp}")
            accB = apsum.tile([128, qw], F32, tag="acc", name=f"accB{branch}{qt2}{hp}")
            for kt in range(nkt):
                s2 = spsum.tile([128, 2 * qw], F32, tag="s2", name=f"s2_{branch}{qt2}{hp}{kt}")
                nc.tensor.matmul(
                    s2[:, 0:qw],
                    ktt[hp][0:64, kt * 128 : (kt + 1) * 128],
                    qtt[hp][0:64, qoff : qoff + qw],
                    start=True,
                    stop=True,
                )
                nc.tensor.matmul(
                    s2[:, qw : 2 * qw],
                    ktt[hp][64:128, kt * 128 : (kt + 1) * 128],
                    qtt[hp][64:128, qoff : qoff + qw],
                    start=True,
                    stop=True,
                    tile_position=(64, 0),
                )
                p2 = p2_p.tile([128, 2 * qw], BF, tag="p2", name=f"p2_{branch}{qt2}{hp}{kt}")
                nc.scalar.activation(p2[:], s2[:], EXP, scale=scale)
                emit_v(kt)
                first = kt == 0
                last = kt == nkt - 1
                # [v_h | 1] lhsT, M=65 -> att rows 0:64 plus the softmax
                # denominator in row 64 of the accumulator
                nc.tensor.matmul(
                    accA[0:65, :],
                    v_sb[kt][:, (2 * hp) * 65 : (2 * hp) * 65 + 65],
                    p2[:, 0:qw],
                    start=first,
                    stop=last,
                    skip_group_check=True,
                )
                nc.tensor.matmul(
                    accB[0:65, :],
                    v_sb[kt][:, (2 * hp + 1) * 65 : (2 * hp + 1) * 65 + 65],
                    p2[:, qw : 2 * qw],
                    start=first,
                    stop=last,
                    skip_group_check=True,
                )
                if filler and kt % 4 == 3:
                    filler.pop(0)()
            while filler:
                filler.pop(0)()
            if "nodiv" in ablate:
                nc.vector.tensor_copy(attT[hp][0:64, :], accA[0:64, :])
                nc.vector.tensor_copy(attT[hp][64:128, :], accB[0:64, :])
                return
            with nc.allow_low_precision(reason="softmax divisor in bf16"):
                rr = rr_p.tile([128, qw], BF, tag="rr", name=f"rrA{branch}{qt2}{hp}")
                nc.vector.reciprocal(rr[64:65, :], accA[64:65, :])
                rr2 = rr_p.tile([128, qw], BF, tag="rr", name=f"rrB{branch}{qt2}{hp}")
                nc.vector.reciprocal(rr2[64:65, :], accB[64:65, :])
            # partition-broadcast of the reciprocal rows: SBUF-source step-0
            # DMAs are rejected, so bounce via a DRAM row (bf16 halves the
            # broadcast descriptor traffic; divisor rounding is negligible)
            rs_d = dram_p.tile([2, qw], BF, tag="rsd", name=f"rsd{branch}{qt2}{hp}")
            nc.sync.dma_start(rs_d[0:1, :], rr[64:65, :])
            nc.sync.dma_start(rs_d[1:2, :], rr2[64:65, :])
            rbA = rb_p.tile([64, qw], BF, tag="rb", name=f"rbA{branch}{qt2}{hp}")
            nc.sync.dma_start(rbA[:], rs_d[0:1, :].to_broadcast((64, qw)))
            rbB = rb_p.tile([64, qw], BF, tag="rb", name=f"rbB{branch}{qt2}{hp}")
            nc.sync.dma_start(rbB[:], rs_d[1:2, :].to_broadcast((64, qw)))
            nc.vector.tensor_mul(attT[hp][0:64, :], accA[0:64, :], rbA[:])
            tmpB = rr_p.tile([64, qw], BF, tag="tmpB", name=f"tmpB{branch}{qt2}{hp}")
            nc.vector.tensor_mul(tmpB[:], accB[0:64, :], rbB[:])
            nc.sync.dma_start(attT[hp][64:128, :], tmpB[:])

        def emit_proj(branch, qt2, attT):
            out_d = xo_d if branch == 0 else yo_d
            for lt in range(qw // 128):
                osb = out_p.tile([128, C], F32, tag="osb", name=f"osb{branch}{qt2}{lt}")
                for half in range(2):
                    pp = apsum.tile([128, CH], F32, tag="acc", name=f"pp{branch}{qt2}{lt}{half}")
                    for ct in range(CT):
                        nc.tensor.matmul(
                            pp[:],
                            attT[ct][:, lt * 128 : (lt + 1) * 128],
                            pw[ct][:, half * CH : (half + 1) * CH],
                            start=(ct == 0),
                            stop=(ct == CT - 1),
                        )
                    nc.vector.tensor_add(
                        osb[:, half * CH : (half + 1) * CH],
                        pp[:],
                        bias[:, half * CH : (half + 1) * CH],
                    )
                row0 = qt2 * qw + lt * 128
                if small_out:
                    if row0 == 0:
                        nc.sync.dma_start(out_d.ap()[0:128, :], osb[:])
                else:
                    nc.sync.dma_start(out_d.ap()[row0 : row0 + 128, :], osb[:])

        def alloc_attT(tagix):
            return [
                att_p.tile([128, qw], BF, tag="attT", name=f"attT{tagix}_{i}")
                for i in range(CT)
            ]

        for st in range(NKT):
            emit_v(st)
        # cross branch: both q-blocks interleaved at head-pair granularity,
        # with the remaining stage-1 K/Q chains spread just-in-time across
        # the 2*CT ACT-paced slots
        nq_cross = L // qw
        attT_cross = [alloc_attT(f"c{q}") for q in range(nq_cross)]
        for hp in range(CT):
            if hp == 0:
                emit_kq(0)
            thunks = kq_chain_thunks(hp + 1) if hp + 1 < CT else []
            fill0 = thunks[: len(thunks) // 2] if nq_cross > 1 else thunks
            rest = thunks[len(fill0) :]
            emit_attn_hp(1, 0, hp, attT_cross[0], filler=fill0)
            for q in range(1, nq_cross):
                emit_attn_hp(1, q, hp, attT_cross[q], filler=rest)
                rest = []
            for th in rest:
                th()
        # projections are deferred one block so they fill the next block's
        # ACT-paced PE slack instead of stalling it
        pending_proj = [(1, q, attT_cross[q]) for q in range(nq_cross)]
        for qt2 in range(N // qw):
            attT = alloc_attT(f"s{qt2}")
            for hp in range(CT):
                emit_attn_hp(0, qt2, hp, attT)
                if pending_proj and hp == 1:
                    emit_proj(*pending_proj.pop(0))
            pending_proj.append((0, qt2, attT))
        for blk in pending_proj:
            emit_proj(*blk)

    nc.compile()
    return nc


def _prep_core_inputs(xb, yb, qkv_wt_bf, proj_wt_bf, proj_b):
    z = np.concatenate([xb, yb], axis=0)  # [S, C]
    zt = np.ascontiguousarray(z.T).astype(ml_dtypes.bfloat16)
    return {
        "z_t": zt,
        "qkv_wt": qkv_wt_bf,
        "proj_wt": proj_wt_bf,
        "proj_b": proj_b.reshape(1, -1).astype(np.float32),
    }


def kernel(x, y, qkv_w, proj_w, proj_b):
    x = np.asarray(x, dtype=np.float32)
    y = np.asarray(y, dtype=np.float32)
    qkv_w = np.asarray(qkv_w, dtype=np.float32)
    proj_w = np.asarray(proj_w, dtype=np.float32)
    proj_b = np.asarray(proj_b, dtype=np.float32)

    qkv_wt_bf = np.ascontiguousarray(qkv_w.T).astype(ml_dtypes.bfloat16)
    proj_wt_bf = np.ascontiguousarray(proj_w.T).astype(ml_dtypes.bfloat16)

    in_maps = [
        _prep_core_inputs(x[b], y[b], qkv_wt_bf, proj_wt_bf, proj_b)
        for b in range(x.shape[0])
    ]
    nc = build_nc()
    res = bass_utils.run_bass_kernel_spmd(nc, in_maps, core_ids=list(range(len(in_maps))))
    x_out = np.stack([res.results[b]["x_out"] for b in range(len(in_maps))])
    y_out = np.stack([res.results[b]["y_out"] for b in range(len(in_maps))])
    return (x_out, y_out)


if __name__ == "__main__":
    import reference

    inputs = {k: np.asarray(v) for k, v in reference.setup_inputs().items()}
    out = kernel(**inputs)
    print("x_out", out[0].shape, "y_out", out[1].shape)


# revision 32
# speedup vs baseline: 9691.9961x; 1.0277x over previous
"""Trainium2 Bass kernel for nn_Attention_927712936452.

Two-branch attention (self branch over x, cross branch of y-queries over
concat(x,y) keys/values), QKV + output projection, H=12 heads of 64.

Distribution: pure data-parallel over batch B=8 across the 8 NeuronCores
(one batch element per core, weights replicated). No collectives.

Per-core layout strategy (all matmul operands bf16, fp32 PSUM accumulate):
  - host supplies z^T [C, S] (c on partitions) so QKV needs no device transpose
  - stage 1 produces q^T/k^T [C, S] (head-pair per 128-row tile) and
    v natural [S, C]
  - scores are computed directly in [k, q] layout: S_T = (k^T)^T-style
    matmul with K=dh=64 contraction; head pair packed into PE row halves
  - softmax skips max-subtraction (scores ~ N(0,1) after 1/8 scale;
    exp is safe in fp32) ; exp on ScalarE with fused scale reads the
    whole [128, 1024] PSUM pair-tile in one instruction
  - softmax denominators ride along in the AV matmuls: v is stored as
    65-wide [v_h | 1] blocks so the M=65 AV matmul leaves the row-sum in
    accumulator row 64; reciprocal on VectorE stays lane-aligned and its
    partition-broadcast bounces through a DRAM row (SBUF-source step-0
    DMAs are rejected); head B's normalized rows are DMA-moved into the
    upper half of the att^T tile
  - normalized att^T tiles feed the output projection directly as lhsT
  - emission order: V first, then the two cross-branch q-blocks
    interleaved at head-pair granularity with the remaining stage-1 Q/K
    chains spread just-in-time across them, then the self branch
"""

import numpy as np

try:
    import concourse.bass as bass  # noqa: F401
except ImportError:
    import sys

    sys.path.insert(0, "/opt/trn_rl_repo")

import ml_dtypes
from contextlib import ExitStack

import concourse.bass as bass
import concourse.tile as tile
from concourse import bacc, bass_utils, mybir

BF = mybir.dt.bfloat16
F32 = mybir.dt.float32
EXP = mybir.ActivationFunctionType.Exp

# Full-size problem constants
B = 8
N_FULL = 1024  # x sequence length (self branch queries/keys)
L_FULL = 1024  # y sequence length (cross branch queries)
C_FULL = 768
H_FULL = 12
DH = 64


def build_nc(C=C_FULL, N=N_FULL, L=L_FULL, qw=512, ablate=(), small_out=False):
    """Build the per-core Bass graph.

    C: model dim (multiple of 128, heads = C//64, head pairs = C//128)
    N: x length, L: y length (each a multiple of qw; qw multiple of 128)
    """
    S = N + L
    CT = C // 128  # head-pair tiles / c-tiles
    NKT = S // 128  # k tiles over full sequence
    NKT_SELF = N // 128  # k tiles for self branch
    CH = C // 2  # proj N-split (PSUM bank limit: <=512 fp32)
    assert CH <= 512 and qw % 128 == 0 and N % qw == 0 and L % qw == 0
    scale = DH ** -0.5

    nc = bacc.Bacc("TRN2", target_bir_lowering=False, debug=False)
    zt_d = nc.dram_tensor("z_t", [C, S], BF, kind="ExternalInput")
    wq_d = nc.dram_tensor("qkv_wt", [C, 3 * C], BF, kind="ExternalInput")
    pw_d = nc.dram_tensor("proj_wt", [C, C], BF, kind="ExternalInput")
    pb_d = nc.dram_tensor("proj_b", [1, C], F32, kind="ExternalInput")
    on = 128 if small_out else N
    ol = 128 if small_out else L
    xo_d = nc.dram_tensor("x_out", [on, C], F32, kind="ExternalOutput")
    yo_d = nc.dram_tensor("y_out", [ol, C], F32, kind="ExternalOutput")

    with tile.TileContext(nc) as tc, ExitStack() as ctx:
        zt_p = ctx.enter_context(tc.tile_pool(name="zt", bufs=CT))
        wq_p = ctx.enter_context(tc.tile_pool(name="wq", bufs=CT))
        qt_p = ctx.enter_context(tc.tile_pool(name="qt", bufs=CT))
        kt_p = ctx.enter_context(tc.tile_pool(name="kt", bufs=CT))
        v_p = ctx.enter_context(tc.tile_pool(name="v", bufs=NKT))
        pw_p = ctx.enter_context(tc.tile_pool(name="pw", bufs=CT))
        misc_p = ctx.enter_context(tc.tile_pool(name="misc", bufs=1))
        p2_p = ctx.enter_context(tc.tile_pool(name="p2", bufs=6))
        att_p = ctx.enter_context(tc.tile_pool(name="attq", bufs=3 * CT))
        rr_p = ctx.enter_context(tc.tile_pool(name="rr", bufs=4))
        rb_p = ctx.enter_context(tc.tile_pool(name="rb", bufs=4))
        out_p = ctx.enter_context(tc.tile_pool(name="osb", bufs=3))
        spsum = ctx.enter_context(tc.tile_pool(name="spsum", bufs=2, space="PSUM"))
        apsum = ctx.enter_context(tc.tile_pool(name="apsum", bufs=4, space="PSUM"))
        dram_p = ctx.enter_context(tc.tile_pool(name="dstage", bufs=2, space="DRAM"))

        # ---- input loads ----
        # chunked so the first V chains / attention can start before the
        # whole input is resident
        zt = [zt_p.tile([128, S], BF, tag="zt", name=f"zt{c}") for c in range(CT)]
        wq = [wq_p.tile([128, 3 * C], BF, tag="wq", name=f"wqt{c}") for c in range(CT)]
        ZCH = S // 4
        for ch in range(4):
            for c in range(CT):
                nc.sync.dma_start(
                    zt[c][:, ch * ZCH : (ch + 1) * ZCH],
                    zt_d.ap()[c * 128 : (c + 1) * 128, ch * ZCH : (ch + 1) * ZCH],
                )
            if ch == 0:
                for c in range(CT):
                    nc.sync.dma_start(
                        wq[c][:, 2 * C : 3 * C],
                        wq_d.ap()[c * 128 : (c + 1) * 128, 2 * C : 3 * C],
                    )
        for c in range(CT):
            nc.sync.dma_start(
                wq[c][:, 0 : 2 * C], wq_d.ap()[c * 128 : (c + 1) * 128, 0 : 2 * C]
            )
        pw = []
        for c in range(CT):
            p1 = pw_p.tile([128, C], BF, tag="pw")
            nc.sync.dma_start(p1[:], pw_d.ap()[c * 128 : (c + 1) * 128, :])
            pw.append(p1)
        bias = misc_p.tile([128, C], F32, tag="bias")
        nc.sync.dma_start(bias[:], pb_d.ap().to_broadcast((128, C)))

        # ---- stage 1: V = z @ Wv ----
        # v-hat layout: every head gets a 65-wide block [v_h | 1]; the ones
        # column lets even heads' AV matmul (M=65) produce the softmax
        # denominator for free in row 64 of the accumulator.
        H = C // DH
        HH = H // 2  # heads per CH half
        v_sb = [v_p.tile([128, H * 65], BF, tag="v", name=f"v{i}") for i in range(NKT)]
        v_emitted = set()

        def emit_v(st):
            if st in v_emitted:
                return
            v_emitted.add(st)
            vh3 = v_sb[st][:].rearrange("p (h e) -> p h e", e=65)
            nc.vector.memset(vh3[:, :, 64:65], 1.0)
            for vn in range(2):
                ps = apsum.tile([128, CH], F32, tag="acc", name=f"vps{st}_{vn}")
                for c in range(CT):
                    nc.tensor.matmul(
                        ps[:],
                        zt[c][:, st * 128 : (st + 1) * 128],
                        wq[c][:, 2 * C + vn * CH : 2 * C + (vn + 1) * CH],
                        start=(c == 0),
                        stop=(c == CT - 1),
                    )
                nc.vector.tensor_copy(
                    vh3[:, vn * HH : (vn + 1) * HH, 0:64],
                    ps[:].rearrange("p (h e) -> p h e", e=64),
                )

        # ---- stage 1: K^T, Q^T  ([d, s] layout, head pair per tile) ----
        qtt = [qt_p.tile([128, S], BF, tag="qt", name=f"qtt{i}") for i in range(CT)]
        ktt = [kt_p.tile([128, S], BF, tag="kt", name=f"ktt{i}") for i in range(CT)]

        def emit_kq_chain(t, n, dst, dbase):
            ps = apsum.tile([128, 512], F32, tag="acc", name=f"kq{t}_{n}_{dbase}")
            for c in range(CT):
                nc.tensor.matmul(
                    ps[:],
                    wq[c][:, dbase + t * 128 : dbase + (t + 1) * 128],
                    zt[c][:, n * 512 : (n + 1) * 512],
                    start=(c == 0),
                    stop=(c == CT - 1),
                )
            nc.vector.tensor_copy(dst[t][:, n * 512 : (n + 1) * 512], ps[:])

        def kq_chain_thunks(t):
            return [
                (lambda t=t, n=n, dst=dst, dbase=dbase: emit_kq_chain(t, n, dst, dbase))
                for n in range(S // 512)
                for dst, dbase in ((ktt, C), (qtt, 0))
            ]

        def emit_kq(t):
            for th in kq_chain_thunks(t):
                th()

        # ---- attention + projection ----
        def emit_attn_hp(branch, qt2, hp, attT, filler=()):
            filler = list(filler)
            nkt = NKT_SELF if branch == 0 else NKT
            qbase = 0 if branch == 0 else N
            qoff = qbase + qt2 * qw
            accA = apsum.tile([128, qw], F32, tag="acc", name=f"accA{branch}{qt2}{hp}")
            accB = apsum.tile([128, qw], F32, tag="acc", name=f"accB{branch}{qt2}{hp}")
            for kt in range(nkt):
                s2 = spsum.tile([128, 2 * qw], F32, tag="s2", name=f"s2_{branch}{qt2}{hp}{kt}")
                nc.tensor.matmul(
                    s2[:, 0:qw],
                    ktt[hp][0:64, kt * 128 : (kt + 1) * 128],
                    qtt[hp][0:64, qoff : qoff + qw],
                    start=True,
                    stop=True,
                )
                nc.tensor.matmul(
                    s2[:, qw : 2 * qw],
                    ktt[hp][64:128, kt * 128 : (kt + 1) * 128],
                    qtt[hp][64:128, qoff : qoff + qw],
                    start=True,
                    stop=True,
                    tile_position=(64, 0),
                )
                p2 = p2_p.tile([128, 2 * qw], BF, tag="p2", name=f"p2_{branch}{qt2}{hp}{kt}")
                nc.scalar.activation(p2[:], s2[:], EXP, scale=scale)
                emit_v(kt)
                first = kt == 0
                last = kt == nkt - 1
                # [v_h | 1] lhsT, M=65 -> att rows 0:64 plus the softmax
                # denominator in row 64 of the accumulator
                nc.tensor.matmul(
                    accA[0:65, :],
                    v_sb[kt][:, (2 * hp) * 65 : (2 * hp) * 65 + 65],
                    p2[:, 0:qw],
                    start=first,
                    stop=last,
                    skip_group_check=True,
                )
                nc.tensor.matmul(
                    accB[0:65, :],
                    v_sb[kt][:, (2 * hp + 1) * 65 : (2 * hp + 1) * 65 + 65],
                    p2[:, qw : 2 * qw],
                    start=first,
                    stop=last,
                    skip_group_check=True,
                )
                if filler and kt % 4 == 3:
                    filler.pop(0)()
            while filler:
                filler.pop(0)()
            if "nodiv" in ablate:
                nc.vector.tensor_copy(attT[hp][0:64, :], accA[0:64, :])
                nc.vector.tensor_copy(attT[hp][64:128, :], accB[0:64, :])
                return
            with nc.allow_low_precision(reason="softmax divisor in bf16"):
                rr = rr_p.tile([128, qw], BF, tag="rr", name=f"rrA{branch}{qt2}{hp}")
                nc.vector.reciprocal(rr[64:65, :], accA[64:65, :])
                rr2 = rr_p.tile([128, qw], BF, tag="rr", name=f"rrB{branch}{qt2}{hp}")
                nc.vector.reciprocal(rr2[64:65, :], accB[64:65, :])
            # partition-broadcast of the reciprocal rows: SBUF-source step-0
            # DMAs are rejected, so bounce via a DRAM row (bf16 halves the
            # broadcast descriptor traffic; divisor rounding is negligible)
            rs_d = dram_p.tile([2, qw], BF, tag="rsd", name=f"rsd{branch}{qt2}{hp}")
            nc.sync.dma_start(rs_d[0:1, :], rr[64:65, :])
            nc.sync.dma_start(rs_d[1:2, :], rr2[64:65, :])
            rbA = rb_p.tile([64, qw], BF, tag="rb", name=f"rbA{branch}{qt2}{hp}")
            nc.sync.dma_start(rbA[:], rs_d[0:1, :].to_broadcast((64, qw)))
            rbB = rb_p.tile([64, qw], BF, tag="rb", name=f"rbB{branch}{qt2}{hp}")
            nc.sync.dma_start(rbB[:], rs_d[1:2, :].to_broadcast((64, qw)))
            # evacuate numerators to SBUF immediately so the PSUM
            # accumulators release before the DRAM-bounce round trip
            numA = rr_p.tile([64, qw], BF, tag="numA", name=f"numA{branch}{qt2}{hp}")
            nc.vector.tensor_copy(numA[:], accA[0:64, :])
            numB = rr_p.tile([64, qw], BF, tag="numB", name=f"numB{branch}{qt2}{hp}")
            nc.vector.tensor_copy(numB[:], accB[0:64, :])
            nc.vector.tensor_mul(attT[hp][0:64, :], numA[:], rbA[:])
            tmpB = rr_p.tile([64, qw], BF, tag="tmpB", name=f"tmpB{branch}{qt2}{hp}")
            nc.vector.tensor_mul(tmpB[:], numB[:], rbB[:])
            nc.sync.dma_start(attT[hp][64:128, :], tmpB[:])

        def emit_proj(branch, qt2, attT):
            out_d = xo_d if branch == 0 else yo_d
            for lt in range(qw // 128):
                osb = out_p.tile([128, C], F32, tag="osb", name=f"osb{branch}{qt2}{lt}")
                for half in range(2):
                    pp = apsum.tile([128, CH], F32, tag="acc", name=f"pp{branch}{qt2}{lt}{half}")
                    for ct in range(CT):
                        nc.tensor.matmul(
                            pp[:],
                            attT[ct][:, lt * 128 : (lt + 1) * 128],
                            pw[ct][:, half * CH : (half + 1) * CH],
                            start=(ct == 0),
                            stop=(ct == CT - 1),
                        )
                    nc.vector.tensor_add(
                        osb[:, half * CH : (half + 1) * CH],
                        pp[:],
                        bias[:, half * CH : (half + 1) * CH],
                    )
                row0 = qt2 * qw + lt * 128
                if small_out:
                    if row0 == 0:
                        nc.sync.dma_start(out_d.ap()[0:128, :], osb[:])
                else:
                    nc.sync.dma_start(out_d.ap()[row0 : row0 + 128, :], osb[:])

        def alloc_attT(tagix):
            return [
                att_p.tile([128, qw], BF, tag="attT", name=f"attT{tagix}_{i}")
                for i in range(CT)
            ]

        for st in range(NKT):
            emit_v(st)
        # cross branch: both q-blocks interleaved at head-pair granularity,
        # with the remaining stage-1 K/Q chains spread just-in-time across
        # the 2*CT ACT-paced slots
        nq_cross = L // qw
        attT_cross = [alloc_attT(f"c{q}") for q in range(nq_cross)]
        for hp in range(CT):
            if hp == 0:
                emit_kq(0)
            thunks = kq_chain_thunks(hp + 1) if hp + 1 < CT else []
            fill0 = thunks[: len(thunks) // 2] if nq_cross > 1 else thunks
            rest = thunks[len(fill0) :]
            emit_attn_hp(1, 0, hp, attT_cross[0], filler=fill0)
            for q in range(1, nq_cross):
                emit_attn_hp(1, q, hp, attT_cross[q], filler=rest)
                rest = []
            for th in rest:
                th()
        # projections are deferred one block so they fill the next block's
        # ACT-paced PE slack instead of stalling it
        pending_proj = [(1, q, attT_cross[q]) for q in range(nq_cross)]
        for qt2 in range(N // qw):
            attT = alloc_attT(f"s{qt2}")
            for hp in range(CT):
                emit_attn_hp(0, qt2, hp, attT)
                if pending_proj and hp == 1:
                    emit_proj(*pending_proj.pop(0))
            pending_proj.append((0, qt2, attT))
        for blk in pending_proj:
            emit_proj(*blk)

    nc.compile()
    return nc


def _prep_core_inputs(xb, yb, qkv_wt_bf, proj_wt_bf, proj_b):
    z = np.concatenate([xb, yb], axis=0)  # [S, C]
    zt = np.ascontiguousarray(z.T).astype(ml_dtypes.bfloat16)
    return {
        "z_t": zt,
        "qkv_wt": qkv_wt_bf,
        "proj_wt": proj_wt_bf,
        "proj_b": proj_b.reshape(1, -1).astype(np.float32),
    }


def kernel(x, y, qkv_w, proj_w, proj_b):
    x = np.asarray(x, dtype=np.float32)
    y = np.asarray(y, dtype=np.float32)
    qkv_w = np.asarray(qkv_w, dtype=np.float32)
    proj_w = np.asarray(proj_w, dtype=np.float32)
    proj_b = np.asarray(proj_b, dtype=np.float32)

    qkv_wt_bf = np.ascontiguousarray(qkv_w.T).astype(ml_dtypes.bfloat16)
    proj_wt_bf = np.ascontiguousarray(proj_w.T).astype(ml_dtypes.bfloat16)

    in_maps = [
        _prep_core_inputs(x[b], y[b], qkv_wt_bf, proj_wt_bf, proj_b)
        for b in range(x.shape[0])
    ]
    nc = build_nc()
    res = bass_utils.run_bass_kernel_spmd(nc, in_maps, core_ids=list(range(len(in_maps))))
    x_out = np.stack([res.results[b]["x_out"] for b in range(len(in_maps))])
    y_out = np.stack([res.results[b]["y_out"] for b in range(len(in_maps))])
    return (x_out, y_out)


if __name__ == "__main__":
    import reference

    inputs = {k: np.asarray(v) for k, v in reference.setup_inputs().items()}
    out = kernel(**inputs)
    print("x_out", out[0].shape, "y_out", out[1].shape)
